# Initial kernel scaffold
#
"""Trainium2 Bass kernel for local (block-sparse) scaled-dot-product attention.

Contract: kernel(**inputs) takes the FULL inputs of the reference
(query/key_in/value [8, 4096, 512] fp32, Wq/Wk/Wv/Wo [512, 512], biases [512])
and returns the FULL output [8, 4096, 512] fp32.

Sharding: data-parallel over batch; batch element b runs on NeuronCore b.

On-chip layout is feature-major ("transposed"): activations live as [feat, t]
so the contraction dim of every matmul is on partitions. The CPU pre-transposes
the inputs/weights (free) and transposes the output back.
"""

import math

import numpy as np
import ml_dtypes

import concourse.bass as bass
import concourse.tile as tile
from concourse import bacc, mybir
from concourse.bass_utils import run_bass_kernel_spmd

# ---- problem constants (hardcoded; must match the reference) ----
B, T, F = 8, 4096, 512
H, DK, DV = 8, 64, 64
CTX = 64          # block size (cq == ck == 64, nb == 64)
NB = T // CTX     # 64 blocks
NEG = -1e20
SCALE = 1.0 / math.sqrt(DK)

TG = 8            # t-groups per core
TT = T // TG      # 512 t positions per group
NB8 = TT // CTX   # 8 blocks per group

# bf16 everywhere on the matmul path (fp32 PSUM accumulation).
DT = mybir.dt.bfloat16
NP_DT = ml_dtypes.bfloat16
F32 = mybir.dt.float32

_CACHED = None  # (nc,) built once


def _flat(ap):
    # [p, a, b] -> [p, a*b] view of a contiguous tile
    return ap.rearrange("p a b -> p (a b)")


def _build_masks():
    """Rank-2 additive masks for the shifted-window attention.

    Scores tile sT[k, q] per block: k in window [64n-32, 64n+96) (j = 0..128),
    q = 0..64.  Invalid pairs get NEG via sum of 2 outer products mj[r] x mi[r].
      mid   : invalid = (j>=96 & q<32) | (j<32 & q>=32)
      first : invalid = (j<32, all q) | (j>=96 & q<32)      (block 0: no prev)
      last  : invalid = (j<32 & q>=32) | (j>=96, all q)     (block 63: no next)
    """
    j = np.arange(128)
    r = np.arange(64)
    mj = np.zeros((3, 2, 128), np.float32)
    mi = np.zeros((3, 2, 64), np.float32)
    # mid
    mj[0, 0] = NEG * (j >= 96)
    mi[0, 0] = (r < 32).astype(np.float32)
    mj[0, 1] = NEG * (j < 32)
    mi[0, 1] = (r >= 32).astype(np.float32)
    # first
    mj[1, 0] = NEG * (j < 32)
    mi[1, 0] = 1.0
    mj[1, 1] = NEG * (j >= 96)
    mi[1, 1] = (r < 32).astype(np.float32)
    # last
    mj[2, 0] = NEG * (j >= 96)
    mi[2, 0] = 1.0
    mj[2, 1] = NEG * (j < 32)
    mi[2, 1] = (r >= 32).astype(np.float32)
    return mj, mi


def _build_nc():
    nc = bacc.Bacc(None, target_bir_lowering=False, debug=False)

    xq = nc.dram_tensor("xq", [F, T], DT, kind="ExternalInput")
    xk = nc.dram_tensor("xk", [F, T], DT, kind="ExternalInput")
    xv = nc.dram_tensor("xv", [F, T], DT, kind="ExternalInput")
    wq = nc.dram_tensor("wq", [F, F], DT, kind="ExternalInput")  # Wq.T
    wk = nc.dram_tensor("wk", [F, F], DT, kind="ExternalInput")  # Wk.T
    wv = nc.dram_tensor("wv", [F, F], DT, kind="ExternalInput")  # Wv.T
    wo = nc.dram_tensor("wo", [F, F], DT, kind="ExternalInput")  # Wo.T
    bq = nc.dram_tensor("bq", [F], F32, kind="ExternalInput")
    bk = nc.dram_tensor("bk", [F], F32, kind="ExternalInput")
    bv = nc.dram_tensor("bv", [F], F32, kind="ExternalInput")
    bo = nc.dram_tensor("bo", [F], F32, kind="ExternalInput")
    mj = nc.dram_tensor("mj", [3, 2, 128], DT, kind="ExternalInput")
    mi = nc.dram_tensor("mi", [3, 2, 64], DT, kind="ExternalInput")
    outd = nc.dram_tensor("out", [F, T], F32, kind="ExternalOutput")

    Exp = mybir.ActivationFunctionType.Exp

    with tile.TileContext(nc) as tc:
        with (
            tc.tile_pool(name="singles", bufs=1) as singles,
            tc.tile_pool(name="xin", bufs=2) as xin,
            tc.tile_pool(name="proj_out", bufs=2) as pqk,
            tc.tile_pool(name="vpool", bufs=2) as vpool,
            tc.tile_pool(name="epool", bufs=3) as epool,
            tc.tile_pool(name="ypool", bufs=2) as ypool,
            tc.tile_pool(name="opool", bufs=2) as opool,
            tc.tile_pool(name="ps_proj", bufs=2, space="PSUM") as ps_proj,
            tc.tile_pool(name="ps_s", bufs=2, space="PSUM") as ps_s,
            tc.tile_pool(name="ps_r", bufs=2, space="PSUM") as ps_r,
            tc.tile_pool(name="ps_o", bufs=2, space="PSUM") as ps_o,
        ):
            # ---- static tiles ----
            wq_t = singles.tile([128, 4, F], DT, tag="wq")
            wk_t = singles.tile([128, 4, F], DT, tag="wk")
            wv_t = singles.tile([128, 4, F], DT, tag="wv")
            wo_t = singles.tile([128, 4, F], DT, tag="wo")
            for wt, wd in ((wq_t, wq), (wk_t, wk), (wv_t, wv), (wo_t, wo)):
                nc.sync.dma_start(out=wt, in_=wd.rearrange("(c p) o -> p c o", p=128))
            bq_t = singles.tile([128, 4], F32, tag="bq")
            bk_t = singles.tile([128, 4], F32, tag="bk")
            bv_t = singles.tile([128, 4], F32, tag="bv")
            bo_t = singles.tile([128, 4], F32, tag="bo")
            for bt, bd in ((bq_t, bq), (bk_t, bk), (bv_t, bv), (bo_t, bo)):
                nc.sync.dma_start(out=bt, in_=bd.rearrange("(c p) -> p c", p=128))
            mj_t = singles.tile([2, 3, 128], DT, tag="mj")
            nc.sync.dma_start(out=mj_t, in_=mj.rearrange("k r j -> r k j"))
            mi_t = singles.tile([2, 3, 64], DT, tag="mi")
            nc.sync.dma_start(out=mi_t, in_=mi.rearrange("k r j -> r k j"))
            ones_col = singles.tile([128, 1], DT, tag="ones_col")
            nc.vector.memset(ones_col, 1.0)
            ones_row = singles.tile([1, 128], DT, tag="ones_row")
            nc.vector.memset(ones_row, 1.0)

            xq_r = xq.rearrange("(c p) t -> p c t", p=128)
            xk_r = xk.rearrange("(c p) t -> p c t", p=128)
            xv_r = xv.rearrange("(c p) t -> p c t", p=128)
            out_r = outd.rearrange("(c p) t -> p c t", p=128)

            for tg in range(TG):
                t0 = tg * TT
                # ---- input loads ----
                xq_s = xin.tile([128, 4, TT], DT, tag="xq")
                nc.sync.dma_start(out=xq_s, in_=xq_r[:, :, t0 : t0 + TT])
                lo, hi = t0 - 32, t0 + TT + 32
                clo, chi = max(lo, 0), min(hi, T)
                xk_s = xin.tile([128, 4, TT + 64], DT, tag="xk")
                xv_s = xin.tile([128, 4, TT + 64], DT, tag="xv")
                for xs, xr in ((xk_s, xk_r), (xv_s, xv_r)):
                    nc.sync.dma_start(
                        out=xs[:, :, clo - lo : chi - lo], in_=xr[:, :, clo:chi]
                    )
                    if clo > lo:
                        nc.vector.memset(xs[:, :, 0 : clo - lo], 0.0)
                    if chi < hi:
                        nc.vector.memset(xs[:, :, TT + 64 - (hi - chi) :], 0.0)

                # ---- q/k projections (feature-major) ----
                qT = pqk.tile([128, 4, TT], DT, tag="qT")
                kT = pqk.tile([128, 4, TT + 64], DT, tag="kT")
                for oc in range(4):
                    ps = ps_proj.tile([128, 512], F32, tag="proj")
                    for fc in range(4):
                        nc.tensor.matmul(
                            ps,
                            lhsT=wq_t[:, fc, oc * 128 : (oc + 1) * 128],
                            rhs=xq_s[:, fc, :],
                            start=(fc == 0),
                            stop=(fc == 3),
                        )
                    nc.vector.tensor_scalar_add(qT[:, oc, :], ps, bq_t[:, oc : oc + 1])
                    ps = ps_proj.tile([128, 512], F32, tag="proj")
                    for fc in range(4):
                        nc.tensor.matmul(
                            ps,
                            lhsT=wk_t[:, fc, oc * 128 : (oc + 1) * 128],
                            rhs=xk_s[:, fc, 0:512],
                            start=(fc == 0),
                            stop=(fc == 3),
                        )
                    nc.vector.tensor_scalar_add(
                        kT[:, oc, 0:512], ps, bk_t[:, oc : oc + 1]
                    )
                    ps2 = ps_proj.tile([128, 64], F32, tag="proj")
                    for fc in range(4):
                        nc.tensor.matmul(
                            ps2,
                            lhsT=wk_t[:, fc, oc * 128 : (oc + 1) * 128],
                            rhs=xk_s[:, fc, 512:576],
                            start=(fc == 0),
                            stop=(fc == 3),
                        )
                    nc.vector.tensor_scalar_add(
                        kT[:, oc, 512:576], ps2, bk_t[:, oc : oc + 1]
                    )

                # ---- v projection (t-major), covering [t0-32, t0+544) ----
                v0 = vpool.tile([128, 5, F], DT, tag="v0")
                for tc5 in range(5):
                    m = 128 if tc5 < 4 else 64
                    ps = ps_proj.tile([128, 512], F32, tag="proj")
                    for fc in range(4):
                        nc.tensor.matmul(
                            ps[0:m, :],
                            lhsT=xv_s[:, fc, 128 * tc5 : 128 * tc5 + m],
                            rhs=wv_t[:, fc, :],
                            start=(fc == 0),
                            stop=(fc == 3),
                        )
                    nc.vector.tensor_copy(out=v0[0:m, tc5, :], in_=ps[0:m, :])
                # shifted copy: v0s covers [t0+32, t0+544), chunk c = rows
                # [64..128) of v0 chunk c plus rows [0..64) of v0 chunk c+1.
                v0s = vpool.tile([128, 4, F], DT, tag="v0s")
                nc.sync.dma_start(out=v0s[0:64, :, :], in_=v0[64:128, 0:4, :])
                nc.sync.dma_start(out=v0s[64:128, :, :], in_=v0[0:64, 1:5, :])

                # ---- attention, per head over 8 blocks ----
                yT = ypool.tile([128, 4, TT], DT, tag="yT")
                oT = None
                for h in range(H):
                    oc, pb = h // 2, (h % 2) * 64
                    sT = ps_s.tile([128, NB8, 64], F32, tag="sT")
                    for n8 in range(NB8):
                        n = tg * NB8 + n8
                        kind = 1 if n == 0 else (2 if n == NB - 1 else 0)
                        nc.tensor.matmul(
                            sT[:, n8, :],
                            lhsT=kT[pb : pb + 64, oc, 64 * n8 : 64 * n8 + 128],
                            rhs=qT[pb : pb + 64, oc, 64 * n8 : 64 * n8 + 64],
                            start=True,
                            stop=False,
                        )
                        nc.tensor.matmul(
                            sT[:, n8, :],
                            lhsT=mj_t[:, kind, :],
                            rhs=mi_t[:, kind, :],
                            start=False,
                            stop=True,
                        )
                    eT = epool.tile([128, NB8, 64], DT, tag="eT")
                    nc.scalar.activation(out=eT, in_=sT, func=Exp, scale=SCALE)
                    sums = ps_r.tile([1, 512], F32, tag="r")
                    nc.tensor.matmul(
                        sums, lhsT=ones_col, rhs=_flat(eT), start=True, stop=True
                    )
                    rs = epool.tile([1, 512], DT, tag="rs")
                    nc.vector.reciprocal(out=rs, in_=sums)
                    bc = ps_r.tile([128, 512], F32, tag="r")
                    nc.tensor.matmul(bc, lhsT=ones_row, rhs=rs, start=True, stop=True)
                    eN = epool.tile([128, NB8, 64], DT, tag="eN")
                    nc.vector.tensor_mul(_flat(eN), _flat(eT), bc)
                    if pb == 0:
                        oT = ps_o.tile([128, 512], F32, tag="oT")
                    for n8 in range(NB8):
                        if n8 % 2 == 0:
                            lhsT = v0[:, n8 // 2, 64 * h : 64 * h + 64]
                        else:
                            lhsT = v0s[:, (n8 - 1) // 2, 64 * h : 64 * h + 64]
                        nc.tensor.matmul(
                            oT[pb : pb + 64, 64 * n8 : 64 * n8 + 64],
                            lhsT=lhsT,
                            rhs=eN[:, n8, :],
                            start=True,
                            stop=True,
                            tile_position=(0, pb),
                        )
                    if pb == 64:
                        nc.vector.tensor_scalar_add(
                            yT[:, oc, :], oT, bv_t[:, oc : oc + 1]
                        )

                # ---- output projection ----
                outsb = opool.tile([128, 4, TT], F32, tag="outsb")
                for oc in range(4):
                    ps = ps_proj.tile([128, 512], F32, tag="proj")
                    for fc in range(4):
                        nc.tensor.matmul(
                            ps,
                            lhsT=wo_t[:, fc, oc * 128 : (oc + 1) * 128],
                            rhs=yT[:, fc, :],
                            start=(fc == 0),
                            stop=(fc == 3),
                        )
                    nc.vector.tensor_scalar_add(
                        outsb[:, oc, :], ps, bo_t[:, oc : oc + 1]
                    )
                nc.sync.dma_start(out=out_r[:, :, t0 : t0 + TT], in_=outsb)

    nc.finalize()
    return nc


def _get_nc():
    global _CACHED
    if _CACHED is None:
        _CACHED = _build_nc()
    return _CACHED


def _prep_in_maps(query, key_in, value, Wq, bq, Wk, bk, Wv, bv, Wo, bo):
    mj, mi = _build_masks()
    shared = {
        "wq": np.ascontiguousarray(Wq.T).astype(NP_DT),
        "wk": np.ascontiguousarray(Wk.T).astype(NP_DT),
        "wv": np.ascontiguousarray(Wv.T).astype(NP_DT),
        "wo": np.ascontiguousarray(Wo.T).astype(NP_DT),
        "bq": np.asarray(bq, np.float32),
        "bk": np.asarray(bk, np.float32),
        "bv": np.asarray(bv, np.float32),
        "bo": np.asarray(bo, np.float32),
        "mj": mj.astype(NP_DT),
        "mi": mi.astype(NP_DT),
    }
    in_maps = []
    for b in range(B):
        in_maps.append(
            {
                "xq": np.ascontiguousarray(np.asarray(query[b]).T).astype(NP_DT),
                "xk": np.ascontiguousarray(np.asarray(key_in[b]).T).astype(NP_DT),
                "xv": np.ascontiguousarray(np.asarray(value[b]).T).astype(NP_DT),
                **shared,
            }
        )
    return in_maps


def run(trace=False, **inputs):
    nc = _get_nc()
    in_maps = _prep_in_maps(**inputs)
    res = run_bass_kernel_spmd(
        nc, in_maps, core_ids=list(range(B)), trace=trace
    )
    out = np.stack(
        [np.asarray(res.results[b]["out"], np.float32).T for b in range(B)]
    )
    return out, res


def kernel(**inputs):
    out, _ = run(trace=False, **inputs)
    return out


# revision 7
# speedup vs baseline: 1.0630x; 1.0630x over previous
"""Trainium2 Bass kernel for local (block-sparse) scaled-dot-product attention.

Contract: kernel(**inputs) takes the FULL inputs of the reference
(query/key_in/value [8, 4096, 512] fp32, Wq/Wk/Wv/Wo [512, 512], biases [512])
and returns the FULL output [8, 4096, 512] fp32.

Sharding: data-parallel over batch; batch element b runs on NeuronCore b.

On-chip layout is feature-major ("transposed"): activations live as [feat, t]
so the contraction dim of every matmul is on partitions. The CPU pre-transposes
the inputs/weights (free) and transposes the output back.
"""

import math

import numpy as np
import ml_dtypes

import concourse.bass as bass
import concourse.tile as tile
from concourse import bacc, mybir
from concourse.bass_utils import run_bass_kernel_spmd

# ---- problem constants (hardcoded; must match the reference) ----
B, T, F = 8, 4096, 512
H, DK, DV = 8, 64, 64
CTX = 64          # block size (cq == ck == 64, nb == 64)
NB = T // CTX     # 64 blocks
NEG = -1e20
SCALE = 1.0 / math.sqrt(DK)

TG = 8            # t-groups per core
TT = T // TG      # 512 t positions per group
NB8 = TT // CTX   # 8 blocks per group

# bf16 everywhere on the matmul path (fp32 PSUM accumulation).
DT = mybir.dt.bfloat16
NP_DT = ml_dtypes.bfloat16
F32 = mybir.dt.float32

_CACHED = None  # (nc,) built once


def _flat(ap):
    # [p, a, b] -> [p, a*b] view of a contiguous tile
    return ap.rearrange("p a b -> p (a b)")


def _build_masks():
    """Rank-2 additive masks for the shifted-window attention.

    Scores tile sT[k, q] per block: k in window [64n-32, 64n+96) (j = 0..128),
    q = 0..64.  Invalid pairs get NEG via sum of 2 outer products mj[r] x mi[r].
      mid   : invalid = (j>=96 & q<32) | (j<32 & q>=32)
      first : invalid = (j<32, all q) | (j>=96 & q<32)      (block 0: no prev)
      last  : invalid = (j<32 & q>=32) | (j>=96, all q)     (block 63: no next)
    """
    j = np.arange(128)
    r = np.arange(64)
    mj = np.zeros((3, 2, 128), np.float32)
    mi = np.zeros((3, 2, 64), np.float32)
    # mid
    mj[0, 0] = NEG * (j >= 96)
    mi[0, 0] = (r < 32).astype(np.float32)
    mj[0, 1] = NEG * (j < 32)
    mi[0, 1] = (r >= 32).astype(np.float32)
    # first
    mj[1, 0] = NEG * (j < 32)
    mi[1, 0] = 1.0
    mj[1, 1] = NEG * (j >= 96)
    mi[1, 1] = (r < 32).astype(np.float32)
    # last
    mj[2, 0] = NEG * (j >= 96)
    mi[2, 0] = 1.0
    mj[2, 1] = NEG * (j < 32)
    mi[2, 1] = (r >= 32).astype(np.float32)
    return mj, mi


def _build_nc(n_iter=1):
    nc = bacc.Bacc(None, target_bir_lowering=False, debug=False)

    xq = nc.dram_tensor("xq", [F, T], DT, kind="ExternalInput")
    xk = nc.dram_tensor("xk", [F, T], DT, kind="ExternalInput")
    xv = nc.dram_tensor("xv", [F, T], DT, kind="ExternalInput")
    wq = nc.dram_tensor("wq", [F, F], DT, kind="ExternalInput")  # Wq.T
    wk = nc.dram_tensor("wk", [F, F], DT, kind="ExternalInput")  # Wk.T
    wv = nc.dram_tensor("wv", [F, F], DT, kind="ExternalInput")  # Wv.T
    wo = nc.dram_tensor("wo", [F, F], DT, kind="ExternalInput")  # Wo.T
    bq = nc.dram_tensor("bq", [F], F32, kind="ExternalInput")
    bk = nc.dram_tensor("bk", [F], F32, kind="ExternalInput")
    bv = nc.dram_tensor("bv", [F], F32, kind="ExternalInput")
    bo = nc.dram_tensor("bo", [F], F32, kind="ExternalInput")
    mj = nc.dram_tensor("mj", [3, 2, 128], DT, kind="ExternalInput")
    mi = nc.dram_tensor("mi", [3, 2, 64], DT, kind="ExternalInput")
    outd = nc.dram_tensor("out", [F, T], F32, kind="ExternalOutput")

    Exp = mybir.ActivationFunctionType.Exp

    with tile.TileContext(nc) as tc:
        with (
            tc.tile_pool(name="singles", bufs=1) as singles,
            tc.tile_pool(name="xin", bufs=2) as xin,
            tc.tile_pool(name="proj_out", bufs=2) as pqk,
            tc.tile_pool(name="vpool", bufs=2) as vpool,
            tc.tile_pool(name="epool", bufs=3) as epool,
            tc.tile_pool(name="ypool", bufs=2) as ypool,
            tc.tile_pool(name="opool", bufs=2) as opool,
            tc.tile_pool(name="ps_proj", bufs=2, space="PSUM") as ps_proj,
            tc.tile_pool(name="ps_s", bufs=2, space="PSUM") as ps_s,
            tc.tile_pool(name="ps_r", bufs=2, space="PSUM") as ps_r,
            tc.tile_pool(name="ps_o", bufs=2, space="PSUM") as ps_o,
        ):
            # ---- static tiles ----
            wq_t = singles.tile([128, 4, F], DT, tag="wq")
            wk_t = singles.tile([128, 4, F], DT, tag="wk")
            wv_t = singles.tile([128, 4, F], DT, tag="wv")
            wo_t = singles.tile([128, 4, F], DT, tag="wo")
            for wt, wd in ((wq_t, wq), (wk_t, wk), (wv_t, wv), (wo_t, wo)):
                nc.sync.dma_start(out=wt, in_=wd.rearrange("(c p) o -> p c o", p=128))
            bq_t = singles.tile([128, 4], F32, tag="bq")
            bk_t = singles.tile([128, 4], F32, tag="bk")
            bv_t = singles.tile([128, 4], F32, tag="bv")
            bo_t = singles.tile([128, 4], F32, tag="bo")
            for bt, bd in ((bq_t, bq), (bk_t, bk), (bv_t, bv), (bo_t, bo)):
                nc.sync.dma_start(out=bt, in_=bd.rearrange("(c p) -> p c", p=128))
            mj_t = singles.tile([2, 3, 128], DT, tag="mj")
            nc.sync.dma_start(out=mj_t, in_=mj.rearrange("k r j -> r k j"))
            mi_t = singles.tile([2, 3, 64], DT, tag="mi")
            nc.sync.dma_start(out=mi_t, in_=mi.rearrange("k r j -> r k j"))
            ones_col = singles.tile([128, 1], DT, tag="ones_col")
            nc.vector.memset(ones_col, 1.0)
            ones_row = singles.tile([1, 128], DT, tag="ones_row")
            nc.vector.memset(ones_row, 1.0)

            xq_r = xq.rearrange("(c p) t -> p c t", p=128)
            xk_r = xk.rearrange("(c p) t -> p c t", p=128)
            xv_r = xv.rearrange("(c p) t -> p c t", p=128)
            out_r = outd.rearrange("(c p) t -> p c t", p=128)

            def emit_group(tg):
                t0 = tg * TT
                # ---- input loads ----
                xq_s = xin.tile([128, 4, TT], DT, tag="xq")
                nc.sync.dma_start(out=xq_s, in_=xq_r[:, :, t0 : t0 + TT])
                lo, hi = t0 - 32, t0 + TT + 32
                clo, chi = max(lo, 0), min(hi, T)
                xk_s = xin.tile([128, 4, TT + 64], DT, tag="xk")
                xv_s = xin.tile([128, 4, TT + 64], DT, tag="xv")
                for xs, xr in ((xk_s, xk_r), (xv_s, xv_r)):
                    nc.sync.dma_start(
                        out=xs[:, :, clo - lo : chi - lo], in_=xr[:, :, clo:chi]
                    )
                    if clo > lo:
                        nc.vector.memset(xs[:, :, 0 : clo - lo], 0.0)
                    if chi < hi:
                        nc.vector.memset(xs[:, :, TT + 64 - (hi - chi) :], 0.0)

                # ---- q/k projections (feature-major) ----
                qT = pqk.tile([128, 4, TT], DT, tag="qT")
                kT = pqk.tile([128, 4, TT + 64], DT, tag="kT")
                for oc in range(4):
                    ps = ps_proj.tile([128, 512], F32, tag="proj")
                    for fc in range(4):
                        nc.tensor.matmul(
                            ps,
                            lhsT=wq_t[:, fc, oc * 128 : (oc + 1) * 128],
                            rhs=xq_s[:, fc, :],
                            start=(fc == 0),
                            stop=(fc == 3),
                        )
                    nc.vector.tensor_scalar_add(qT[:, oc, :], ps, bq_t[:, oc : oc + 1])
                    ps = ps_proj.tile([128, 512], F32, tag="proj")
                    for fc in range(4):
                        nc.tensor.matmul(
                            ps,
                            lhsT=wk_t[:, fc, oc * 128 : (oc + 1) * 128],
                            rhs=xk_s[:, fc, 0:512],
                            start=(fc == 0),
                            stop=(fc == 3),
                        )
                    nc.vector.tensor_scalar_add(
                        kT[:, oc, 0:512], ps, bk_t[:, oc : oc + 1]
                    )
                    ps2 = ps_proj.tile([128, 64], F32, tag="proj")
                    for fc in range(4):
                        nc.tensor.matmul(
                            ps2,
                            lhsT=wk_t[:, fc, oc * 128 : (oc + 1) * 128],
                            rhs=xk_s[:, fc, 512:576],
                            start=(fc == 0),
                            stop=(fc == 3),
                        )
                    nc.vector.tensor_scalar_add(
                        kT[:, oc, 512:576], ps2, bk_t[:, oc : oc + 1]
                    )

                # ---- v projection (t-major), covering [t0-32, t0+544) ----
                v0 = vpool.tile([128, 5, F], DT, tag="v0")
                for tc5 in range(5):
                    m = 128 if tc5 < 4 else 64
                    ps = ps_proj.tile([128, 512], F32, tag="proj")
                    for fc in range(4):
                        nc.tensor.matmul(
                            ps[0:m, :],
                            lhsT=xv_s[:, fc, 128 * tc5 : 128 * tc5 + m],
                            rhs=wv_t[:, fc, :],
                            start=(fc == 0),
                            stop=(fc == 3),
                        )
                    nc.vector.tensor_copy(out=v0[0:m, tc5, :], in_=ps[0:m, :])
                # shifted copy: v0s covers [t0+32, t0+544), chunk c = rows
                # [64..128) of v0 chunk c plus rows [0..64) of v0 chunk c+1.
                v0s = vpool.tile([128, 4, F], DT, tag="v0s")
                nc.sync.dma_start(out=v0s[0:64, :, :], in_=v0[64:128, 0:4, :])
                nc.sync.dma_start(out=v0s[64:128, :, :], in_=v0[0:64, 1:5, :])

                # ---- attention, per head over 8 blocks ----
                yT = ypool.tile([128, 4, TT], DT, tag="yT")
                oT = None
                for h in range(H):
                    oc, pb = h // 2, (h % 2) * 64
                    sT = ps_s.tile([128, NB8, 64], F32, tag="sT")
                    for n8 in range(NB8):
                        n = tg * NB8 + n8
                        kind = 1 if n == 0 else (2 if n == NB - 1 else 0)
                        nc.tensor.matmul(
                            sT[:, n8, :],
                            lhsT=kT[pb : pb + 64, oc, 64 * n8 : 64 * n8 + 128],
                            rhs=qT[pb : pb + 64, oc, 64 * n8 : 64 * n8 + 64],
                            start=True,
                            stop=False,
                        )
                        nc.tensor.matmul(
                            sT[:, n8, :],
                            lhsT=mj_t[:, kind, :],
                            rhs=mi_t[:, kind, :],
                            start=False,
                            stop=True,
                        )
                    eT = epool.tile([128, NB8, 64], DT, tag="eT")
                    nc.scalar.activation(out=eT, in_=sT, func=Exp, scale=SCALE)
                    sums = ps_r.tile([1, 512], F32, tag="r")
                    nc.tensor.matmul(
                        sums, lhsT=ones_col, rhs=_flat(eT), start=True, stop=True
                    )
                    rs = epool.tile([1, 512], DT, tag="rs")
                    with nc.allow_low_precision(reason="bf16 softmax denominators"):
                        nc.vector.reciprocal(out=rs, in_=sums)
                    bc = ps_r.tile([128, 512], F32, tag="r")
                    nc.tensor.matmul(bc, lhsT=ones_row, rhs=rs, start=True, stop=True)
                    eN = epool.tile([128, NB8, 64], DT, tag="eN")
                    nc.vector.tensor_mul(_flat(eN), _flat(eT), bc)
                    if pb == 0:
                        oT = ps_o.tile([128, 512], F32, tag="oT")
                    for n8 in range(NB8):
                        if n8 % 2 == 0:
                            lhsT = v0[:, n8 // 2, 64 * h : 64 * h + 64]
                        else:
                            lhsT = v0s[:, (n8 - 1) // 2, 64 * h : 64 * h + 64]
                        nc.tensor.matmul(
                            oT[pb : pb + 64, 64 * n8 : 64 * n8 + 64],
                            lhsT=lhsT,
                            rhs=eN[:, n8, :],
                            start=True,
                            stop=True,
                            tile_position=(0, pb),
                        )
                    if pb == 64:
                        nc.vector.tensor_scalar_add(
                            yT[:, oc, :], oT, bv_t[:, oc : oc + 1]
                        )

                # ---- output projection ----
                outsb = opool.tile([128, 4, TT], F32, tag="outsb")
                for oc in range(4):
                    ps = ps_proj.tile([128, 512], F32, tag="proj")
                    for fc in range(4):
                        nc.tensor.matmul(
                            ps,
                            lhsT=wo_t[:, fc, oc * 128 : (oc + 1) * 128],
                            rhs=yT[:, fc, :],
                            start=(fc == 0),
                            stop=(fc == 3),
                        )
                    nc.vector.tensor_scalar_add(
                        outsb[:, oc, :], ps, bo_t[:, oc : oc + 1]
                    )
                nc.sync.dma_start(out=out_r[:, :, t0 : t0 + TT], in_=outsb)

            if n_iter == 1:
                for tg in range(TG):
                    emit_group(tg)
            else:
                with tc.For_i(0, n_iter, 1):
                    for tg in range(TG):
                        emit_group(tg)

    nc.finalize()
    return nc


def _get_nc(n_iter=1):
    global _CACHED
    if _CACHED is None:
        _CACHED = {}
    if n_iter not in _CACHED:
        _CACHED[n_iter] = _build_nc(n_iter)
    return _CACHED[n_iter]


def _prep_in_maps(query, key_in, value, Wq, bq, Wk, bk, Wv, bv, Wo, bo):
    mj, mi = _build_masks()
    shared = {
        "wq": np.ascontiguousarray(Wq.T).astype(NP_DT),
        "wk": np.ascontiguousarray(Wk.T).astype(NP_DT),
        "wv": np.ascontiguousarray(Wv.T).astype(NP_DT),
        "wo": np.ascontiguousarray(Wo.T).astype(NP_DT),
        "bq": np.asarray(bq, np.float32),
        "bk": np.asarray(bk, np.float32),
        "bv": np.asarray(bv, np.float32),
        "bo": np.asarray(bo, np.float32),
        "mj": mj.astype(NP_DT),
        "mi": mi.astype(NP_DT),
    }
    in_maps = []
    for b in range(B):
        in_maps.append(
            {
                "xq": np.ascontiguousarray(np.asarray(query[b]).T).astype(NP_DT),
                "xk": np.ascontiguousarray(np.asarray(key_in[b]).T).astype(NP_DT),
                "xv": np.ascontiguousarray(np.asarray(value[b]).T).astype(NP_DT),
                **shared,
            }
        )
    return in_maps


def run(trace=False, **inputs):
    nc = _get_nc()
    in_maps = _prep_in_maps(**inputs)
    res = run_bass_kernel_spmd(
        nc, in_maps, core_ids=list(range(B)), trace=trace
    )
    out = np.stack(
        [np.asarray(res.results[b]["out"], np.float32).T for b in range(B)]
    )
    return out, res


def kernel(**inputs):
    out, _ = run(trace=False, **inputs)
    return out
